# revision 11
# baseline (speedup 1.0000x reference)
"""FFM layer (nn_FFM_Layer) Trainium2 Bass kernel.

Reference computation (B=4096, 13 dense fields, 26 sparse fields with vocab
1000 each, FIELD_NUM=39, K=16):

    idx        = sparse + offsets                      # [B, 26] global ids
    first      = w0 + dense @ w[:13] + sum_j w[idx]    # [B, 1]
    field_f    = einsum('bd,dfk', dense, v[:13]) + sum_j v[idx]   # [B,39,16]
    s          = field_f.sum(1)                        # [B, 16]
    second     = 0.5*(||s||^2 - sum_fk field_f^2)      # [B]
    out        = first + second[:, None]

Strategy (data-parallel over batch, 8 cores x 512 samples, no collectives):
  * Host packs an augmented table V_AUG [26013, 640] f32:
      cols [0:624]  = v.reshape(26013, 39*16)
      col  624      = w[:, 0]   (+ w0 folded into rows of sparse table 0,
                                 which every sample hits exactly once)
      cols [625:640]= 0         (pad so each row is 2560 B, %256 == 0)
  * Each core runs dma_gather (SWDGE, mlp ucode lib) over its 512*26 rows:
    one gathered row brings both the v-row and its w contribution, so a
    single accumulation chain produces field_f AND the sparse w-sum.
    Gathers are sample-chunk-major: 4 calls (7+7+6+6 fields x 128 samples)
    per 128-sample chunk, so each chunk's FM epilogue and output DMA
    overlap the next chunk's gathers.  Q7 descriptor generation (~9 ns/row)
    is the critical path; the SWDGE descriptor ring caps one call at
    ~1024 descriptors (1536+ wedges the exec unit).
  * Dense contribution comes from a [13,128]x[13,640] PE matmul per chunk
    (dense^T is prepared host-side), which also adds dense @ w[:13] via
    col 624; the PSUM->SBUF copy doubles as the accumulator init.
  * FM identity epilogue per chunk: ACT Square+accum for the norms
    (InstTensorTensorReduce wedges the exec unit on this HW path), DVE
    tree for the 39-field sum.

Roofline: 4096*26 rows x 2560 B = 266 MB of gathers, 33.3 MB/core; both
the DMA bus (~360 GB/s -> ~95 us) and Q7 desc-gen (~9 ns/row -> ~120 us)
sit near the wall.
"""

import sys

if "/opt/trn_rl_repo" not in sys.path:
    sys.path.insert(0, "/opt/trn_rl_repo")

import numpy as np

import concourse.bacc as bacc
import concourse.bass as bass
import concourse.tile as tile
from concourse import mybir
from concourse.bass_utils import run_bass_kernel_spmd

# Problem constants (hardcoded per harness contract)
B = 4096
N_DENSE = 13
N_SPARSE = 26
FEAT_PER_SPARSE = 1000
FIELD_NUM = 39
FEATURE_NUM = 26013
K = 16
N_CORES = 8
BC = B // N_CORES          # 512 samples per core
ROW = 640                  # padded row: 624 v + 1 w + 15 zeros (2560 B)
VCOLS = FIELD_NUM * K      # 624
P = 128
SCHUNKS = BC // P          # 4 sample chunks of 128 per core
# per-chunk gather calls: field groups (sum 26), each call = nf*128 idxs
FGROUPS = [7, 7, 6, 6]
IDX_COLS_SC = N_SPARSE * P // 16   # 208 idx cols per sample chunk

F32 = mybir.dt.float32
I16 = mybir.dt.int16


def build_program():
    """Build + compile the single-core SPMD bass program."""
    nc = bacc.Bacc("TRN2", target_bir_lowering=False, debug=False,
                   num_swdge_queues=4)

    vaug_t = nc.dram_tensor("vaug", [FEATURE_NUM, ROW], F32, kind="ExternalInput")
    dense_t = nc.dram_tensor("dense_t", [N_DENSE, BC], F32, kind="ExternalInput")
    idxs_t = nc.dram_tensor("idxs", [P, SCHUNKS * IDX_COLS_SC], I16,
                            kind="ExternalInput")
    out_t = nc.dram_tensor("out", [P, SCHUNKS], F32, kind="ExternalOutput")

    with tile.TileContext(nc) as tc:
        with (
            tc.tile_pool(name="main", bufs=1) as main,
            tc.tile_pool(name="gath", bufs=5) as gath,
            tc.tile_pool(name="fold", bufs=2) as fold,
            tc.tile_pool(name="small", bufs=2) as small,
            tc.tile_pool(name="psum", bufs=2, space="PSUM") as psum,
        ):
            # per-sample-chunk idx tiles so the first gather starts early
            idx_sbs = []
            for c in range(SCHUNKS):
                t = main.tile([P, IDX_COLS_SC], I16, tag=f"idx{c}")
                nc.sync.dma_start(
                    t[:], idxs_t[:, c * IDX_COLS_SC : (c + 1) * IDX_COLS_SC]
                )
                idx_sbs.append(t)
            vaug13 = main.tile([N_DENSE, ROW], F32)
            nc.sync.dma_start(vaug13[:], vaug_t[0:N_DENSE, :])
            dt_sb = main.tile([N_DENSE, BC], F32)
            nc.sync.dma_start(dt_sb[:], dense_t[:])

            res = main.tile([P, SCHUNKS], F32)

            for c in range(SCHUNKS):
                # dense part -> PSUM; the PSUM->SBUF copy inits the chunk acc
                ps = psum.tile([P, ROW], F32, tag="ps")
                nc.tensor.matmul(
                    out=ps[:, 0:512],
                    lhsT=dt_sb[:, c * P : (c + 1) * P],
                    rhs=vaug13[:, 0:512],
                    start=True,
                    stop=True,
                )
                nc.tensor.matmul(
                    out=ps[:, 512:ROW],
                    lhsT=dt_sb[:, c * P : (c + 1) * P],
                    rhs=vaug13[:, 512:ROW],
                    start=True,
                    stop=True,
                )
                acc = main.tile([P, ROW], F32, tag=f"acc{c}")
                nc.scalar.copy(acc[:], ps[:])

                icol = 0
                for gi, nf in enumerate(FGROUPS):
                    n_idx = nf * P
                    g = gath.tile([P, 7, ROW], F32, tag="g")
                    nc.gpsimd.dma_gather(
                        g[:, :nf, :],
                        vaug_t[:],
                        idx_sbs[c][:, icol : icol + n_idx // 16],
                        n_idx,
                        n_idx,
                        ROW,
                        single_packet=False,
                        queue_num=(c * len(FGROUPS) + gi) % 4,
                    )
                    icol += n_idx // 16
                    # wide tree fold: nf cols -> 1 col, then one acc add
                    t1 = fold.tile([P, 3, ROW], F32, tag="t1")
                    t2 = fold.tile([P, ROW], F32, tag="t2")
                    nc.vector.tensor_add(t1[:], g[:, 0:3, :], g[:, 3:6, :])
                    nc.vector.tensor_add(t2[:], t1[:, 0, :], t1[:, 1, :])
                    nc.vector.tensor_add(t2[:], t2[:], t1[:, 2, :])
                    if nf == 7:
                        nc.vector.tensor_add(t2[:], t2[:], g[:, 6, :])
                    nc.vector.tensor_add(acc[:], acc[:], t2[:])

                # --- FM identity epilogue for this chunk ---
                blk = acc[:, 0:VCOLS]             # [128, 624] = field_f
                sq = fold.tile([P, VCOLS], F32, tag="sq")
                q = small.tile([P, 1], F32, tag="q")
                nc.scalar.activation(
                    sq[:], blk, mybir.ActivationFunctionType.Square,
                    accum_out=q[:],
                )
                # s-tree: sum 39 fields of 16 -> st[:, 0:16]
                st = fold.tile([P, 320], F32, tag="st")
                # 39 = 19 pairs + 1 leftover -> 20 fields in st
                nc.vector.tensor_add(st[:, 0:304], blk[:, 0:304], blk[:, 304:608])
                nc.scalar.copy(st[:, 304:320], blk[:, 608:624])
                nc.vector.tensor_add(st[:, 0:160], st[:, 0:160], st[:, 160:320])
                nc.vector.tensor_add(st[:, 0:80], st[:, 0:80], st[:, 80:160])
                nc.vector.tensor_add(st[:, 0:32], st[:, 0:32], st[:, 32:64])
                nc.vector.tensor_add(st[:, 0:16], st[:, 0:16], st[:, 16:32])
                nc.vector.tensor_add(st[:, 0:16], st[:, 0:16], st[:, 64:80])
                s2 = small.tile([P, 16], F32, tag="s2")
                snorm = small.tile([P, 1], F32, tag="snorm")
                nc.scalar.activation(
                    s2[:], st[:, 0:16], mybir.ActivationFunctionType.Square,
                    accum_out=snorm[:],
                )
                diff = small.tile([P, 1], F32, tag="diff")
                nc.vector.tensor_tensor(
                    out=diff[:], in0=snorm[:], in1=q[:],
                    op=mybir.AluOpType.subtract,
                )
                # out = 0.5*diff + (w-sum incl. w0 and dense first-order)
                nc.scalar.activation(
                    res[:, c : c + 1],
                    diff[:],
                    mybir.ActivationFunctionType.Identity,
                    bias=acc[:, VCOLS : VCOLS + 1],
                    scale=0.5,
                )
                nc.sync.dma_start(out_t[:, c : c + 1], res[:, c : c + 1])

    nc.compile()
    return nc


def prep_inputs(dense_inputs, sparse_inputs, w0, w, v):
    """Host-side shard/pack: build per-core in_maps."""
    dense = np.asarray(dense_inputs, np.float32)
    sparse = np.asarray(sparse_inputs)
    w0 = np.asarray(w0, np.float32)
    w = np.asarray(w, np.float32)
    v = np.asarray(v, np.float32)

    vaug = np.zeros((FEATURE_NUM, ROW), np.float32)
    vaug[:, :VCOLS] = v.reshape(FEATURE_NUM, VCOLS)
    vaug[:, VCOLS] = w[:, 0]
    # fold w0 into sparse table 0 (each sample hits it exactly once)
    vaug[N_DENSE : N_DENSE + FEAT_PER_SPARSE, VCOLS] += w0[0]

    offs = N_DENSE + FEAT_PER_SPARSE * np.arange(N_SPARSE, dtype=np.int64)
    gidx = (sparse.astype(np.int64) + offs[None, :]).astype(np.int16)  # [B, 26]

    in_maps = []
    for core in range(N_CORES):
        sl = slice(core * BC, (core + 1) * BC)
        dt = np.ascontiguousarray(dense[sl].T)          # [13, 512]
        idxc = gidx[sl]                                 # [512, 26]
        buf = np.zeros((P, SCHUNKS * IDX_COLS_SC), np.int16)
        off_c = 0
        for c in range(SCHUNKS):
            rows = idxc[c * P : (c + 1) * P]            # [128, 26]
            fbase = 0
            for nf in FGROUPS:
                n = nf * P
                # call order: i = f_local*128 + p  ->  row idx[p, fbase+f]
                seg = np.ascontiguousarray(
                    rows[:, fbase : fbase + nf].T
                ).reshape(-1)                           # [nf*128]
                wrapped = seg.reshape(n // 16, 16).T    # [16, n/16]
                buf[:, off_c : off_c + n // 16] = np.tile(wrapped, (8, 1))
                fbase += nf
                off_c += n // 16
        in_maps.append({"vaug": vaug, "dense_t": dt, "idxs": buf})
    return in_maps


_NC_CACHE = None


def kernel(dense_inputs, sparse_inputs, w0, w, v):
    global _NC_CACHE
    if _NC_CACHE is None:
        _NC_CACHE = build_program()
    nc = _NC_CACHE
    in_maps = prep_inputs(dense_inputs, sparse_inputs, w0, w, v)
    res = run_bass_kernel_spmd(nc, in_maps, core_ids=list(range(N_CORES)))
    outs = []
    for r in res.results:
        o = r["out"]                                    # [128, 4]
        outs.append(np.ascontiguousarray(o.T).reshape(BC, 1))
    return np.concatenate(outs, axis=0).astype(np.float32)


# revision 13
# speedup vs baseline: 1.2189x; 1.2189x over previous
"""FFM layer (nn_FFM_Layer) Trainium2 Bass kernel.

Reference computation (B=4096, 13 dense fields, 26 sparse fields with vocab
1000 each, FIELD_NUM=39, K=16):

    idx        = sparse + offsets                      # [B, 26] global ids
    first      = w0 + dense @ w[:13] + sum_j w[idx]    # [B, 1]
    field_f    = einsum('bd,dfk', dense, v[:13]) + sum_j v[idx]   # [B,39,16]
    s          = field_f.sum(1)                        # [B, 16]
    second     = 0.5*(||s||^2 - sum_fk field_f^2)      # [B]
    out        = first + second[:, None]

Strategy (data-parallel over batch, 8 cores x 512 samples, no collectives):
  * Host packs an augmented table V_AUG [26013, 640] f32:
      cols [0:624]  = v.reshape(26013, 39*16)
      col  624      = w[:, 0]   (+ w0 folded into rows of sparse table 0,
                                 which every sample hits exactly once)
      cols [625:640]= 0         (pad so each row is 2560 B, %256 == 0)
  * Each core runs dma_gather (SWDGE, mlp ucode lib) over its 512*26 rows:
    one gathered row brings both the v-row and its w contribution, so a
    single accumulation chain produces field_f AND the sparse w-sum.
    Gathers are sample-chunk-major: 4 calls (7+7+6+6 fields x 128 samples)
    per 128-sample chunk, so each chunk's FM epilogue and output DMA
    overlap the next chunk's gathers.  Q7 descriptor generation (~9 ns/row)
    is the critical path; the SWDGE descriptor ring caps one call at
    ~1024 descriptors (1536+ wedges the exec unit).
  * Dense contribution comes from a [13,128]x[13,640] PE matmul per chunk
    (dense^T is prepared host-side), which also adds dense @ w[:13] via
    col 624; the PSUM->SBUF copy doubles as the accumulator init.
  * FM identity epilogue per chunk: ACT Square+accum for the norms
    (InstTensorTensorReduce wedges the exec unit on this HW path), DVE
    tree for the 39-field sum.

Roofline: 4096*26 rows x 2560 B = 266 MB of gathers, 33.3 MB/core; both
the DMA bus (~360 GB/s -> ~95 us) and Q7 desc-gen (~9 ns/row -> ~120 us)
sit near the wall.
"""

import sys

if "/opt/trn_rl_repo" not in sys.path:
    sys.path.insert(0, "/opt/trn_rl_repo")

import numpy as np

import concourse.bacc as bacc
import concourse.bass as bass
import concourse.tile as tile
from concourse import mybir
from concourse.bass_utils import run_bass_kernel_spmd

# Problem constants (hardcoded per harness contract)
B = 4096
N_DENSE = 13
N_SPARSE = 26
FEAT_PER_SPARSE = 1000
FIELD_NUM = 39
FEATURE_NUM = 26013
K = 16
N_CORES = 8
BC = B // N_CORES          # 512 samples per core
ROW = 640                  # padded row: 624 v + 1 w + 15 zeros (2560 B)
VCOLS = FIELD_NUM * K      # 624
P = 128
SCHUNKS = BC // P          # 4 sample chunks of 128 per core
# per-chunk gather calls: field groups (sum 26), each call = nf*128 idxs
FGROUPS = [7, 7, 6, 6]
FGROUPS_LAST = [7, 7, 6, 4, 2]
IDX_COLS_SC = N_SPARSE * P // 16   # 208 idx cols per sample chunk

F32 = mybir.dt.float32
I16 = mybir.dt.int16


def build_program():
    """Build + compile the single-core SPMD bass program."""
    nc = bacc.Bacc("TRN2", target_bir_lowering=False, debug=False,
                   num_swdge_queues=2)

    vaug_t = nc.dram_tensor("vaug", [FEATURE_NUM, ROW], F32, kind="ExternalInput")
    dense_t = nc.dram_tensor("dense_t", [N_DENSE, BC], F32, kind="ExternalInput")
    idxs_t = nc.dram_tensor("idxs", [P, SCHUNKS * IDX_COLS_SC], I16,
                            kind="ExternalInput")
    out_t = nc.dram_tensor("out", [P, SCHUNKS], F32, kind="ExternalOutput")

    with tile.TileContext(nc) as tc:
        with (
            tc.tile_pool(name="main", bufs=1) as main,
            tc.tile_pool(name="gath", bufs=6) as gath,
            tc.tile_pool(name="fold", bufs=2) as fold,
            tc.tile_pool(name="small", bufs=2) as small,
            tc.tile_pool(name="psum", bufs=2, space="PSUM") as psum,
        ):
            # per-sample-chunk idx tiles so the first gather starts early
            idx_sbs = []
            for c in range(SCHUNKS):
                t = main.tile([P, IDX_COLS_SC], I16, tag=f"idx{c}")
                nc.sync.dma_start(
                    t[:], idxs_t[:, c * IDX_COLS_SC : (c + 1) * IDX_COLS_SC]
                )
                idx_sbs.append(t)
            vaug13 = main.tile([N_DENSE, ROW], F32)
            nc.sync.dma_start(vaug13[:], vaug_t[0:N_DENSE, :])
            dt_sb = main.tile([N_DENSE, BC], F32)
            nc.sync.dma_start(dt_sb[:], dense_t[:])

            res = main.tile([P, SCHUNKS], F32)

            for c in range(SCHUNKS):
                # dense part -> PSUM; the PSUM->SBUF copy inits the chunk acc
                ps = psum.tile([P, ROW], F32, tag="ps")
                nc.tensor.matmul(
                    out=ps[:, 0:512],
                    lhsT=dt_sb[:, c * P : (c + 1) * P],
                    rhs=vaug13[:, 0:512],
                    start=True,
                    stop=True,
                )
                nc.tensor.matmul(
                    out=ps[:, 512:ROW],
                    lhsT=dt_sb[:, c * P : (c + 1) * P],
                    rhs=vaug13[:, 512:ROW],
                    start=True,
                    stop=True,
                )
                acc = main.tile([P, ROW], F32, tag=f"acc{c}")
                nc.scalar.copy(acc[:], ps[:])

                icol = 0
                fgroups = FGROUPS if c < SCHUNKS - 1 else FGROUPS_LAST
                for gi, nf in enumerate(fgroups):
                    n_idx = nf * P
                    g = gath.tile([P, 7, ROW], F32, tag="g")
                    nc.gpsimd.dma_gather(
                        g[:, :nf, :],
                        vaug_t[:],
                        idx_sbs[c][:, icol : icol + n_idx // 16],
                        n_idx,
                        n_idx,
                        ROW,
                        single_packet=False,
                        queue_num=(c * len(FGROUPS) + gi) % 2,
                    )
                    icol += n_idx // 16
                    # wide tree fold: nf cols -> 1 col, then one acc add
                    if nf >= 6:
                        t1 = fold.tile([P, 3, ROW], F32, tag="t1")
                        t2 = fold.tile([P, ROW], F32, tag="t2")
                        nc.vector.tensor_add(t1[:], g[:, 0:3, :], g[:, 3:6, :])
                        nc.vector.tensor_add(t2[:], t1[:, 0, :], t1[:, 1, :])
                        nc.vector.tensor_add(t2[:], t2[:], t1[:, 2, :])
                        if nf == 7:
                            nc.vector.tensor_add(t2[:], t2[:], g[:, 6, :])
                        nc.vector.tensor_add(acc[:], acc[:], t2[:])
                    elif nf == 4:
                        t1 = fold.tile([P, 3, ROW], F32, tag="t1")
                        t2 = fold.tile([P, ROW], F32, tag="t2")
                        nc.vector.tensor_add(t1[:, 0:2, :], g[:, 0:2, :], g[:, 2:4, :])
                        nc.vector.tensor_add(t2[:], t1[:, 0, :], t1[:, 1, :])
                        nc.vector.tensor_add(acc[:], acc[:], t2[:])
                    else:  # nf == 2
                        t2 = fold.tile([P, ROW], F32, tag="t2")
                        nc.vector.tensor_add(t2[:], g[:, 0, :], g[:, 1, :])
                        nc.vector.tensor_add(acc[:], acc[:], t2[:])

                # --- FM identity epilogue for this chunk ---
                blk = acc[:, 0:VCOLS]             # [128, 624] = field_f
                sq = fold.tile([P, VCOLS], F32, tag="sq")
                q = small.tile([P, 1], F32, tag="q")
                nc.scalar.activation(
                    sq[:], blk, mybir.ActivationFunctionType.Square,
                    accum_out=q[:],
                )
                # s = sum over the 39 fields: strided reduce of [128,16,39]
                st = fold.tile([P, 16], F32, tag="st")
                blk_kf = acc[:, 0:VCOLS].rearrange("p (f k) -> p k f", k=16)
                nc.vector.tensor_reduce(
                    out=st[:], in_=blk_kf, op=mybir.AluOpType.add,
                    axis=mybir.AxisListType.X,
                )
                s2 = small.tile([P, 16], F32, tag="s2")
                snorm = small.tile([P, 1], F32, tag="snorm")
                nc.scalar.activation(
                    s2[:], st[:], mybir.ActivationFunctionType.Square,
                    accum_out=snorm[:],
                )
                diff = small.tile([P, 1], F32, tag="diff")
                nc.vector.tensor_tensor(
                    out=diff[:], in0=snorm[:], in1=q[:],
                    op=mybir.AluOpType.subtract,
                )
                # out = 0.5*diff + (w-sum incl. w0 and dense first-order)
                nc.scalar.activation(
                    res[:, c : c + 1],
                    diff[:],
                    mybir.ActivationFunctionType.Identity,
                    bias=acc[:, VCOLS : VCOLS + 1],
                    scale=0.5,
                )
                nc.sync.dma_start(out_t[:, c : c + 1], res[:, c : c + 1])

    nc.compile()
    return nc


def prep_inputs(dense_inputs, sparse_inputs, w0, w, v):
    """Host-side shard/pack: build per-core in_maps."""
    dense = np.asarray(dense_inputs, np.float32)
    sparse = np.asarray(sparse_inputs)
    w0 = np.asarray(w0, np.float32)
    w = np.asarray(w, np.float32)
    v = np.asarray(v, np.float32)

    vaug = np.zeros((FEATURE_NUM, ROW), np.float32)
    vaug[:, :VCOLS] = v.reshape(FEATURE_NUM, VCOLS)
    vaug[:, VCOLS] = w[:, 0]
    # fold w0 into sparse table 0 (each sample hits it exactly once)
    vaug[N_DENSE : N_DENSE + FEAT_PER_SPARSE, VCOLS] += w0[0]

    offs = N_DENSE + FEAT_PER_SPARSE * np.arange(N_SPARSE, dtype=np.int64)
    gidx = (sparse.astype(np.int64) + offs[None, :]).astype(np.int16)  # [B, 26]

    in_maps = []
    for core in range(N_CORES):
        sl = slice(core * BC, (core + 1) * BC)
        dt = np.ascontiguousarray(dense[sl].T)          # [13, 512]
        idxc = gidx[sl]                                 # [512, 26]
        buf = np.zeros((P, SCHUNKS * IDX_COLS_SC), np.int16)
        off_c = 0
        for c in range(SCHUNKS):
            rows = idxc[c * P : (c + 1) * P]            # [128, 26]
            fbase = 0
            for nf in (FGROUPS if c < SCHUNKS - 1 else FGROUPS_LAST):
                n = nf * P
                # call order: i = f_local*128 + p  ->  row idx[p, fbase+f]
                seg = np.ascontiguousarray(
                    rows[:, fbase : fbase + nf].T
                ).reshape(-1)                           # [nf*128]
                wrapped = seg.reshape(n // 16, 16).T    # [16, n/16]
                buf[:, off_c : off_c + n // 16] = np.tile(wrapped, (8, 1))
                fbase += nf
                off_c += n // 16
        in_maps.append({"vaug": vaug, "dense_t": dt, "idxs": buf})
    return in_maps


_NC_CACHE = None


def kernel(dense_inputs, sparse_inputs, w0, w, v):
    global _NC_CACHE
    if _NC_CACHE is None:
        _NC_CACHE = build_program()
    nc = _NC_CACHE
    in_maps = prep_inputs(dense_inputs, sparse_inputs, w0, w, v)
    res = run_bass_kernel_spmd(nc, in_maps, core_ids=list(range(N_CORES)))
    outs = []
    for r in res.results:
        o = r["out"]                                    # [128, 4]
        outs.append(np.ascontiguousarray(o.T).reshape(BC, 1))
    return np.concatenate(outs, axis=0).astype(np.float32)
